# revision 9
# baseline (speedup 1.0000x reference)
"""Trainium2 Bass kernel for AffinityLoss (nn_AffinityLoss_70875550318911), v6.

Math: loss = mean over (n, a, b, l) of BCEWithLogits(aff_map, lb_map) where
aff_map[n,a,b,l] = sum_c lu[n,c,a,l]*lu[n,c,b,l] over 3x3 unfold positions a,b.

Reformulation: pairs (a,b) sharing relative offset d=(di,dj) share one
correlation map D_d[p] = sum_c logits[c,p]*logits[c,p+d]; by symmetry only 13
offsets are needed. Border multiplicities factorize into row weights rw(y)
times col weights cw(x). With lt = ln(sigmoid(-D)):

  contrib_d = sum_{r,x} (rw*cw) * (-lt)  +  (-rw*cw*m) * D   (m = label match)
  loss = sum_d sym_d * contrib_d / (n * 81 * 382^2)

Engine split (per core: 96 owned image rows = 2 batches x 48):
  layout: 114 partitions = (c=19, group=6), free = [16, 384] product tiles
  (group rows x W), logits stored as 18 rows x 384 (16 owned + 2 halo), bf16.
  - DVE:    shifted products for ~9.4 offsets (bf16 TT, 2x mode)
  - Pool:   products for ~2.6 offsets via scalar_tensor_tensor (eff 0.6,
            fp8 out), plus the 13 j0 STTs: (D * 1) * MCW with accum_out
            giving the x-sum per row directly (no PE reduction needed)
  - ACT:    q0 = Square(L) (fp8 out); sigmoid(-D); ln(u) -> LT
  - PE:     c-sum as indicator matmuls: bf16 offsets 16 steps; fp8 offsets 8
            DoubleRow steps (half cost); 13 LT row-sum matmuls with rw as the
            stationary into PTL[5, 384] grouped by dj (host applies cw(x));
            2 tiny matmuls reduce the j0 accums; warmup matmuls beat the
            PE p-state ramp during the DMA head.

Host sums PTL (weighted by -cw) and the j0 accums over cores, applies the
global scale.
"""
import os
import numpy as np
import ml_dtypes

NCORES = 8
N, C, H, W = 2, 19, 384, 384
KS = 3
BAND = H // NCORES            # 48 owned rows per core per batch
NGRP = 6                      # groups: (batch=2) x (row-block=3)
GR = 16                       # owned rows per group
TRG = GR + 2                  # rows stored per group (owned + halo)
PART = C * NGRP               # 114 partitions
FREE = TRG * W                # 6912 data elems per partition
PADF = FREE + 4               # +2 pad each side
OROWS = NGRP * GR             # 96 output rows (partitions of D)
MULF = GR * W                 # 6144 elems per offset multiply
NOFF = 13
IW = OROWS + (GR - 1) * NGRP  # 186 sliding-indicator columns
NDJ = 5                       # dj in -2..2 -> 5 PTL rows

# (di, dj, sym): di >= 0; for di == 0 only dj >= 0. sym 2 covers (-di,-dj).
OFFSETS = [(0, 0, 1.0), (0, 1, 2.0), (0, 2, 2.0),
           (1, -2, 2.0), (1, -1, 2.0), (1, 0, 2.0), (1, 1, 2.0), (1, 2, 2.0),
           (2, -2, 2.0), (2, -1, 2.0), (2, 0, 2.0), (2, 1, 2.0), (2, 2, 2.0)]

def _env_ints(name, default):
    return [int(x) for x in os.environ.get(name, default).split(",") if x != ""]

POOL_OFFS = _env_ints("AFF_POOL_OFFS", "5,10")   # full products on Pool (fp8)
SPLIT_OFF = int(os.environ.get("AFF_SPLIT", "12"))   # bf16, split DVE/Pool
SPLIT_S = int(os.environ.get("AFF_SPLIT_S", "16"))    # s < SPLIT_S on DVE
SEQ = _env_ints("AFF_SEQ", "1,2,3,4,5,0,6,7,8,9,10,11,12")
NWARM = int(os.environ.get("AFF_WARM", "10"))
J0POOL = _env_ints("AFF_J0POOL", "1,2,3,4,5,0,6,7,8")  # j0 TTs on Pool
PHASES = _env_ints("AFF_PHASES", "5,9,13")  # ACT sigmoid/ln phase ends
WORK_BUFS = int(os.environ.get("AFF_WORK_BUFS", "6"))
PSUM_BUFS = int(os.environ.get("AFF_PSUM_BUFS", "5"))

BF16 = ml_dtypes.bfloat16
FP8 = ml_dtypes.float8_e4m3

_PROGRAM = None
LAST_RESULTS = None  # BassKernelResults of the most recent run (for profiling)


def _mult_weight(d: int, p: int, size: int = H) -> int:
    """Number of 3x3 window anchors pairing pixel p with p+d along one axis."""
    lo, hi = max(0, -d), 2 - max(d, 0)
    lo2, hi2 = max(lo, p - (size - KS)), min(hi, p)
    return max(0, hi2 - lo2 + 1)


# lg DMA piece boundaries (columns of the [PART, PADF] logits band)
PIECES = [0, 2 + 2 * W, 2 + 5 * W, 2 + 8 * W, 2 + 13 * W, PADF]


def _prod_chunks(shift: int, max_chunks: int):
    """s-ranges [(s0, s1)) such that chunk i's in1 columns fit within DMA
    piece boundaries, allowing the product to start before the whole band
    has landed. max_chunks=1 -> single chunk."""
    if max_chunks <= 1:
        return [(0, GR)]
    cuts = set()
    for b in PIECES[1:-1]:
        s = (b - 2 - shift) // W
        if 0 < s < GR:
            cuts.add(int(s))
    cuts = sorted(cuts)[: max_chunks - 1]
    bounds = [0] + cuts + [GR]
    return [(a, b) for a, b in zip(bounds[:-1], bounds[1:]) if b > a]


def _build_program():
    import concourse.tile as tile
    from concourse import bacc, mybir
    from concourse.alu_op_type import AluOpType
    from contextlib import ExitStack

    bf = mybir.dt.bfloat16
    f32 = mybir.dt.float32
    f8 = mybir.dt.float8e4
    A = AluOpType
    AF = mybir.ActivationFunctionType
    DR = mybir.MatmulPerfMode.DoubleRow

    nc = bacc.Bacc("TRN2", target_bir_lowering=False, debug=False,
                   num_devices=NCORES)

    lg_d = nc.dram_tensor("lg", [PART, PADF], bf, kind="ExternalInput")
    wts_d = nc.dram_tensor("wts", [OROWS, NOFF * W + NOFF * NDJ], bf,
                           kind="ExternalInput")
    ind_d = nc.dram_tensor("ind", [PART, IW], bf, kind="ExternalInput")
    indd_d = nc.dram_tensor("indd", [PART, GR * OROWS], f8,
                            kind="ExternalInput")
    out = nc.dram_tensor("out", [NDJ, 2 * W], f32, kind="ExternalOutput")

    fp8_offs = set(POOL_OFFS) | {0}

    with ExitStack() as ctx:
        tc = ctx.enter_context(tile.TileContext(nc))
        singles = ctx.enter_context(tc.tile_pool(name="singles", bufs=1))
        work = ctx.enter_context(tc.tile_pool(name="work", bufs=WORK_BUFS))
        pipe = ctx.enter_context(tc.tile_pool(name="pipe", bufs=4))
        dcp = ctx.enter_context(tc.tile_pool(name="dcp", bufs=3))
        psumd = ctx.enter_context(tc.tile_pool(
            name="psumd", bufs=PSUM_BUFS, space="PSUM"))
        psum_ptl = ctx.enter_context(tc.tile_pool(name="psum_ptl", bufs=1,
                                                  space="PSUM"))
        psum_acc = ctx.enter_context(tc.tile_pool(name="psum_acc", bufs=1,
                                                  space="PSUM"))
        psum_wu = ctx.enter_context(tc.tile_pool(name="psum_wu", bufs=1,
                                                 space="PSUM"))

        LG = singles.tile([PART, PADF], bf, name="LG")
        WTS = singles.tile([OROWS, NOFF * W + NOFF * NDJ], bf, name="WTS")
        MCW = [WTS[:, q * W:(q + 1) * W] for q in range(NOFF)]
        RWB = [WTS[:, NOFF * W + q * NDJ:NOFF * W + (q + 1) * NDJ]
               for q in range(NOFF)]
        IND = singles.tile([PART, IW], bf)
        INDD = singles.tile([PART, GR, OROWS], f8)
        ONESB = singles.tile([OROWS, 1], bf)
        WUS = singles.tile([128, 256], bf)
        LT = [singles.tile([OROWS, W], bf, name=f"LT{q}") for q in range(NOFF)]
        RES = singles.tile([NDJ, 2 * W], f32)

        # --- input DMAs: lg in 4 pieces across queues; aux tensors behind
        qd_for_piece = [nc.sync, nc.scalar, nc.sync, nc.scalar,
                        nc.sync]
        for qd, (lo, hi) in zip(qd_for_piece, zip(PIECES[:-1], PIECES[1:])):
            qd.dma_start(LG[:, lo:hi], lg_d[:, lo:hi])
        nc.sync.dma_start(IND[:], ind_d[:])
        nc.sync.dma_start(INDD[:, :, :], indd_d[:, :])
        nc.gpsimd.dma_start(WTS[:, 0:NOFF * W], wts_d[:, 0:NOFF * W])
        nc.gpsimd.dma_start(WTS[:, NOFF * W:], wts_d[:, NOFF * W:])

        # --- memsets + PE warmup chain (ramps the p-state during DMA head)
        nc.vector.memset(WUS[:], 0.03125)
        nc.vector.memset(ONESB[:], 1.0)
        WUP = psum_wu.tile([1, 256], f32)
        for _ in range(NWARM):
            nc.tensor.matmul(WUP[:], WUS[:, 0:1], WUS[:], start=True,
                             stop=True, skip_group_check=True)

        PTL = psum_ptl.tile([NDJ, W], f32)
        PT0 = psum_acc.tile([1, W], f32)

        from concourse.tile import add_dep_helper
        act_seq = []

        def _act(*args, **kw):
            inst = nc.scalar.activation(*args, **kw)
            act_seq.append(inst)
            return inst

        # --- products -------------------------------------------------------
        prods = {}

        def emit_dve_prod(q, max_chunks):
            di, dj, _ = OFFSETS[q]
            shift = di * W + dj
            s_hi = SPLIT_S if q == SPLIT_OFF else GR
            pr = work.tile([PART, GR, W], bf, tag="prod")
            for s0, s1 in _prod_chunks(shift, max_chunks):
                s1 = min(s1, s_hi)
                if s1 <= s0:
                    continue
                nc.vector.tensor_tensor(
                    pr[:, s0:s1, :], LG[:, 2 + s0 * W:2 + s1 * W],
                    LG[:, 2 + shift + s0 * W:2 + shift + s1 * W], A.mult)
            prods[q] = pr

        def emit_pool_prod(q, max_chunks, s_lo=0, dt=f8):
            di, dj, _ = OFFSETS[q]
            shift = di * W + dj
            pr = singles.tile([PART, GR, W], dt, name=f"poolprod{q}")
            for s0, s1 in _prod_chunks(shift, max_chunks):
                s0 = max(s0, s_lo)
                if s1 <= s0:
                    continue
                nc.gpsimd.tensor_tensor(
                    pr[:, s0:s1, :], LG[:, 2 + s0 * W:2 + s1 * W],
                    LG[:, 2 + shift + s0 * W:2 + shift + s1 * W], A.mult)
            return pr

        def emit_act_prod(q, max_chunks):
            pr = singles.tile([PART, GR, W], f8, name="actprod0")
            for s0, s1 in _prod_chunks(0, max_chunks):
                _act(pr[:, s0:s1, :],
                     LG[:, 2 + s0 * W:2 + s1 * W], AF.Square)
            prods[q] = pr

        # ACT product (q0) and Pool products emitted up-front so the Pool/ACT
        # queues fill from t~0 and PE's first real matmuls come early.
        emit_act_prod(0, 4)
        for q in POOL_OFFS:
            prods[q] = emit_pool_prod(q, 3)
        if SPLIT_OFF >= 0 and SPLIT_S < GR:
            split_pool_pr = emit_pool_prod(SPLIT_OFF, 1, s_lo=SPLIT_S, dt=bf)

        # --- per-offset csum + nonlinearity + weighted sums ----------------
        emm_cnt = [0]
        j0t = {}
        dct = {}
        utt = {}

        def emit_lt_emm(q):
            nc.tensor.matmul(PTL[:], RWB[q], LT[q][:],
                             start=(emm_cnt[0] == 0),
                             stop=(emm_cnt[0] == NOFF - 1),
                             skip_group_check=True)
            nc.tensor.matmul(PT0[:], ONESB[:], j0t[q][:],
                             start=(emm_cnt[0] == 0),
                             stop=(emm_cnt[0] == NOFF - 1),
                             skip_group_check=True)
            emm_cnt[0] += 1

        def emit_offset(q):
            D = psumd.tile([OROWS, W], f32, tag="D")
            if q in fp8_offs:
                for s2 in range(GR // 2):
                    nc.tensor.matmul(
                        D[:], INDD[:, 2 * s2:2 * s2 + 2, :],
                        prods[q][:, 2 * s2:2 * s2 + 2, :],
                        start=(s2 == 0), stop=(s2 == GR // 2 - 1),
                        perf_mode=DR)
            else:
                base = (GR - 1) * NGRP
                for s in range(GR):
                    mv = prods[q]
                    if q == SPLIT_OFF and s >= SPLIT_S:
                        mv = split_pool_pr
                    nc.tensor.matmul(
                        D[:], IND[:, base - NGRP * s:base - NGRP * s + OROWS],
                        mv[:, s, :], start=(s == 0), stop=(s == GR - 1))
            UT = singles.tile([OROWS, W], bf, name=f"UT{q}")
            _act(UT[:], D[:], AF.Sigmoid, scale=-1.0)
            DC = dcp.tile([OROWS, W], bf, tag="dc")
            _act(DC[:], D[:], AF.Copy)
            dct[q] = DC
            utt[q] = UT

        phase_groups = []
        lo = 0
        for hi in PHASES:
            phase_groups.append(SEQ[lo:hi])
            lo = hi

        for gi, grp in enumerate(phase_groups):
            base = sum(len(g) for g in phase_groups[:gi])
            for i, q in enumerate(grp):
                if q not in prods:
                    pos = base + i
                    emit_dve_prod(q, 4 if pos <= 2 else 2)
                emit_offset(q)
            # B phase: lns for this group (one table swap per boundary)
            for q in grp:
                _act(LT[q][:], utt[q][:], AF.Ln)
            # j0 TTs for this group (DC is ready); Pool or DVE per J0POOL
            for q in grp:
                j0 = pipe.tile([OROWS, W], bf, tag="j0")
                eng = nc.gpsimd if q in J0POOL else nc.vector
                eng.tensor_tensor(j0[:], dct[q][:], MCW[q], A.mult)
                j0t[q] = j0
            for q in grp:
                emit_lt_emm(q)

        # pin ACT order so the scheduler can't interleave table phases
        for i in range(1, len(act_seq)):
            add_dep_helper(act_seq[i].ins, act_seq[i - 1].ins, sync=False,
                           reason="ACT table-phase order")

        # --- final reductions + output -------------------------------------
        nc.vector.tensor_copy(RES[:, 0:W], PTL[:])
        nc.vector.tensor_copy(RES[0:1, W:2 * W], PT0[:])
        nc.sync.dma_start(out[:], RES[:])
    nc.compile()
    return nc


def _host_inputs(logits: np.ndarray, labels: np.ndarray):
    logits = np.asarray(logits, dtype=np.float32)
    labels = np.asarray(labels)
    lg_bf = logits.astype(BF16)                      # (n, c, h, w)

    cw = np.zeros((NDJ, W), dtype=np.float32)
    for j, dj in enumerate(range(-2, 3)):
        cw[j] = [_mult_weight(dj, px, W) for px in range(W)]
    wy_tab = np.array([[_mult_weight(d, py, H) for py in range(H)]
                      for d in range(3)], dtype=np.float32)

    ind = np.zeros((PART, IW), dtype=BF16)
    ind[np.arange(PART), (GR - 1) * NGRP + np.arange(PART) % NGRP] = 1.0

    # explicit per-step indicators for DoubleRow csum: block s is the
    # [PART, OROWS] one-hot with 1 at column 6*s + p%6
    indd = np.zeros((PART, GR, OROWS), dtype=FP8)
    for s in range(GR):
        indd[np.arange(PART), s, NGRP * s + np.arange(PART) % NGRP] = 1.0
    indd = indd.reshape(PART, GR * OROWS)

    in_maps = []
    for k in range(NCORES):
        m = {"ind": ind, "indd": indd}
        # logits band: [c*6+g, 2 + r*384 + x], g = b*3+gb,
        # rows y = k*48 + gb*16 + r for r in 0..17 (zero-padded past H)
        ga = np.zeros((PART, PADF), dtype=BF16)
        for b in range(N):
            for gb in range(3):
                g = b * 3 + gb
                y0 = k * BAND + gb * GR
                rows = min(TRG, H - y0)
                blk = np.zeros((C, TRG, W), dtype=BF16)
                blk[:, :rows, :] = lg_bf[b, :, y0:y0 + rows, :]
                ga[g::NGRP, 2:2 + FREE] = blk.reshape(C, FREE)
        m["lg"] = ga

        # D partition 6s+g <-> (batch b, image row y = k*48 + gb*16 + s)
        rw = np.zeros((OROWS, NOFF), dtype=np.float32)
        for q, (di, dj, sym) in enumerate(OFFSETS):
            for g in range(NGRP):
                b, gb = divmod(g, 3)
                ys = k * BAND + gb * GR + np.arange(GR)
                rw[g::NGRP, q] = sym * wy_tab[di, ys]

        wts = np.zeros((OROWS, NOFF * W + NOFF * NDJ), dtype=np.float32)
        # rw stationary blocks: column dj+2 of block q holds rw_q
        for q, (di, dj, sym) in enumerate(OFFSETS):
            wts[:, NOFF * W + q * NDJ + (dj + 2)] = rw[:, q]

        # mcw_q = -(rw * cw * [labels match]) in the permuted row order
        for q, (di, dj, sym) in enumerate(OFFSETS):
            mc = np.zeros((OROWS, W), dtype=np.float32)
            x0, x1 = max(0, -dj), W - max(dj, 0)
            for g in range(NGRP):
                b, gb = divmod(g, 3)
                ys = k * BAND + gb * GR + np.arange(GR)
                val = ys + di < H
                yv = ys[val]
                mm = (labels[b, yv, x0:x1] == labels[b, yv + di, x0 + dj:x1 + dj])
                blk = np.zeros((GR, W), dtype=np.float32)
                blk[val, x0:x1] = -(mm * cw[dj + 2][x0:x1])
                mc[g::NGRP] = blk
            wts[:, q * W:(q + 1) * W] = mc * rw[:, q:q + 1]
        m["wts"] = wts.astype(BF16)
        in_maps.append(m)
    return in_maps


def kernel(logits: np.ndarray, labels: np.ndarray) -> np.ndarray:
    global _PROGRAM, LAST_RESULTS
    from concourse.bass_utils import run_bass_kernel_spmd

    if _PROGRAM is None:
        _PROGRAM = _build_program()

    in_maps = _host_inputs(logits, labels)
    trace = bool(int(os.environ.get("AFF_TRACE", "0")))
    results = run_bass_kernel_spmd(
        _PROGRAM, in_maps, core_ids=list(range(NCORES)), trace=trace)
    LAST_RESULTS = results

    cw = np.zeros((NDJ, W), dtype=np.float64)
    for j, dj in enumerate(range(-2, 3)):
        cw[j] = [_mult_weight(dj, px, W) for px in range(W)]

    total = 0.0
    for r in results.results:
        o = np.asarray(r["out"], dtype=np.float64)
        total += float((o[:, 0:W] * (-cw)).sum())   # softplus part
        total += float(o[0, W:2 * W].sum())         # linear (j0) part
    Lwin = (H - KS + 1) * (W - KS + 1)
    return np.float32(total / (N * KS**4 * Lwin))
